# revision 75
# baseline (speedup 1.0000x reference)
"""HYV3Attention (qkv proj + qk-RMSNorm + neox RoPE + causal GQA attention +
o_proj) on 8 Trainium2 NeuronCores.

Sharding: tensor-parallel across heads. Core c owns q heads 4c..4c+3 and kv
head c (GQA group c), i.e. 768 of the 6144 qkv_proj rows and 512 of the 4096
o_proj columns. Each core produces a full [T, HIDDEN] partial of the output
(o_proj contracts only over its own heads); the host sums the 8 partials.
No collectives.

Per-core device kernel (all matmuls bf16, f32 accumulation), tuned so the PE
issues back-to-back (measured ~216ns per 512-col matmul at full clock):
  1. qkvT = w_local @ hidden.T -> [768, 2048] "feature-on-partition", weights
     DMA'd in per-(k-piece, m-tile) 256KB slices so the first matmul starts
     after <1MB has landed.
  2. RMSNorm via sum-of-squares matmul with an all-ones [128,128] lhsT (lands
     the reduce already broadcast across partitions; same PE cost as a 1-row
     reduce); r = (ssq*scl+eps')^-1/2 computed as exp(-0.5*ln(.)) — two
     ScalarE table ops, no sqrt and no (slow, ~6cyc/elem) DVE reciprocal.
     RoPE as elementwise multiplies against host-precomputed cos/sin tables
     with the norm weights folded in (the half-rotation comes from a PE
     permutation matmul); 1/sqrt(HEAD_DIM) and eps fold into ln's scale/bias.
     Each chunk's norm work is spread through the NEXT chunk's m-loop (and the
     last chunk reorders its m-tiles k-first) so DVE/ScalarE latency hides
     under the qkv matmul stream.
  3. Scores S.T tile [k=128, q=512] = kT.T @ qT ; softmax without max
     subtraction (RMS-normed scores are bounded by sqrt(128)); exp on ScalarE
     straight out of PSUM; causal masking by multiplying the 4 diagonal
     k-tiles with 0/1 masks; attnT accumulates in PSUM with lhsT = v-tiles;
     softmax denominators accumulate Σ_kt p on the DVE (in-place adds) with a
     single all-ones matmul + exp(-ln(dn)) + multiply at flush time.
  4. out_partial = attn_flat @ w_o_slice.T with lhsT = attnT tiles (attnT
     aliases the dead q rows of qkvT). o_proj matmuls of the previous q-chunk
     are interleaved into the exp-gated k-tile loops to fill PE bubbles;
     outputs accumulate in [128, 2, 512] two-bank PSUM pairs (one copy + one
     256KB DMA each, alternating DVE/ScalarE), written bf16 and summed f32 on
     the host.
"""
import os

import numpy as np
import ml_dtypes

import concourse.bass as bass
import concourse.mybir as mybir
import concourse.tile as tile
from concourse.bass import ts
from concourse.masks import make_identity

BF16 = ml_dtypes.bfloat16
F32 = mybir.dt.float32
BF = mybir.dt.bfloat16

T = 2048
HIDDEN = 4096
D = 128  # head dim
N_CORES = 8
HPC = 4  # q heads per core
KO = HIDDEN // 128  # 32 contraction tiles for qkv proj
MQKV = (HPC + 2) * D // 128  # 6 partition tiles of qkvT (4 q heads, k, v)
NCH = T // 512  # 4 free-dim chunks of 512
NKT = T // 128  # 16 k tiles
ROPE_THETA = 10000.0
RMS_EPS = 1e-5

# ---------------------------------------------------------------------------
# Workaround: this walrus build rejects Drain instructions carrying more than
# one sem-wait ("Too many sync wait commands"). Split the Tile tail drain into
# one Drain per outstanding logical proc, each with a single wait.
_PATCHED = False


def _patch_tile_tail():
    global _PATCHED
    if _PATCHED:
        return
    _PATCHED = True
    import concourse.tile as ctile
    from concourse.vector_clock import ScopedClock, VectorClock

    def _drain_and_barrier_split(self, tick_clock, wait_clock):
        gc = tick_clock.global_clock
        n = len(gc)
        for p in range(n):
            if gc[p] == 0:
                continue
            partial = VectorClock([gc[i] if i == p else 0 for i in range(n)])
            d = self.nc.sync.drain()
            wait_clock.add_sem_waits(d.ins, ScopedClock({None: partial}))
        self.nc.all_engine_barrier()
        assert self.sems is not None
        popped = self.nc._tile_sem_poison_stack.pop()
        assert popped is self._sem_poison
        self.nc.clear_and_free_semaphores(list(self.sems.allocated().values()))
        self.nc.all_engine_barrier()

    ctile.TileContext._drain_and_barrier = _drain_and_barrier_split


# ---------------------------------------------------------------------------
# Optional NTFF tracing support (KERNEL_TRACE=1): register the axon profile
# hook that this image's antenv lacks, and neuter the S3 artifact upload.
def _enable_tracing():
    import sys
    import types

    if "antenv.axon_hooks" not in sys.modules:
        holder = {"hook": None}
        mod = types.ModuleType("antenv.axon_hooks")
        mod.set_axon_ntff_profile_hook = lambda h: holder.__setitem__("hook", h)
        mod.get_axon_ntff_profile_hook = lambda: holder["hook"]
        sys.modules["antenv.axon_hooks"] = mod
        from trn_agent_boot.trn_boot import _ntff_profile_via_ctypes

        mod.set_axon_ntff_profile_hook(
            _ntff_profile_via_ctypes("/opt/axon/libaxon_pjrt.so")
        )
    import concourse.bass_utils as bu

    bu.upload_artifacts = lambda tmpdir: f"file://{tmpdir}"


# ---------------------------------------------------------------------------
def build_nc():
    _patch_tile_tail()
    nc = bass.Bass()

    # hiddenT is staged chunk-major ([chunk, hidden, 512]) so each (chunk,
    # k-piece) DMA reads one contiguous DRAM region (aggregatable packets)
    hiddenT = nc.dram_tensor("hiddenT", [NCH, HIDDEN, 512], BF, kind="ExternalInput")
    # w_qkvT is staged m-tile-major ([m, ko, p, 128]) so each (k-piece, m)
    # DMA reads one contiguous 256KB DRAM region
    w_qkvT = nc.dram_tensor(
        "w_qkvT", [MQKV, KO, 128, 128], BF, kind="ExternalInput"
    )
    w_oT = nc.dram_tensor("w_oT", [HPC * D, HIDDEN], BF, kind="ExternalInput")
    ropeAq = nc.dram_tensor("ropeAq", [D, T], BF, kind="ExternalInput")
    ropeBq = nc.dram_tensor("ropeBq", [D, T], BF, kind="ExternalInput")
    ropeAk = nc.dram_tensor("ropeAk", [D, T], BF, kind="ExternalInput")
    ropeBk = nc.dram_tensor("ropeBk", [D, T], BF, kind="ExternalInput")
    maskT = nc.dram_tensor("maskT", [128, 4, 512], BF, kind="ExternalInput")
    swapmat_d = nc.dram_tensor("swapmat", [128, 128], BF, kind="ExternalInput")
    outp = nc.dram_tensor("outp", [T, HIDDEN], BF, kind="ExternalOutput")

    with tile.TileContext(nc) as tc:
        with (
            tc.tile_pool(name="const", bufs=1) as pconst,
            tc.tile_pool(name="qkv", bufs=1) as pqkv,
            tc.tile_pool(name="aux", bufs=2) as paux,
            tc.tile_pool(name="qk_rope", bufs=1) as pqk,
        ):
            # ---- constants -------------------------------------------------
            identity = pconst.tile([128, 128], BF)
            make_identity(nc, identity)
            ones_mat = pconst.tile([128, 128], BF)
            nc.vector.memset(ones_mat, 1.0)
            # RMS eps with the q/k score scales folded in (see norm_chunk)
            bias_k = pconst.tile([128, 1], F32)
            nc.vector.memset(bias_k, RMS_EPS)
            bias_q = pconst.tile([128, 1], F32)
            nc.vector.memset(bias_q, RMS_EPS * D)
            swapmat = pconst.tile([128, 128], BF)

            qkvT_sb = pqkv.tile([128, MQKV, T], BF)
            # attnT aliases the q-head rows of qkvT: every norm read of a
            # (hh, chunk) slice completes before that slice's flush write
            # (flush depends on qs which depends on the same norm reads), and
            # disjoint slices carry no dependency. Saves 2MB of SBUF.
            attnT_sb = qkvT_sb[:, 0:HPC, :]

            tabAq = pqk.tile([D, T], BF)
            tabBq = pqk.tile([D, T], BF)
            tabAk = pqk.tile([D, T], BF)
            tabBk = pqk.tile([D, T], BF)
            mask_sb = pqk.tile([128, 4, 512], BF)
            qs_sb = pqk.tile([128, HPC, T], BF)  # roped+scaled q per head
            ks_sb = pqk.tile([128, T], BF)  # roped+scaled k
            v_sb = pqk.tile([128, NKT, D], BF)  # v in [token, d] layout

            # ---- phase A: qkvT = w_local @ hidden.T, with the norms, rope
            # and v transposes interleaved per 512-token chunk so the DVE /
            # ScalarE work hides under the qkv matmul stream ----------------
            with (
                tc.tile_pool(name="wq", bufs=1) as pw1,
                tc.tile_pool(name="hid", bufs=2) as ph,
                tc.tile_pool(name="ps_qkv", bufs=2, space="PSUM") as ps_qkv,
                tc.tile_pool(name="ps_na", bufs=2, space="PSUM") as ps_na,
            ):
                NKP = 4  # k-dim DMA pieces
                KH = KO // NKP
                wT = w_qkvT.rearrange("m ko p c -> m p ko c")
                hT = hiddenT.rearrange("c (ko p) t -> c p ko t", p=128)
                # The weight load is sliced per (k-piece, m-tile): the very
                # first matmul chain (m=0) only waits for the four 256KB m=0
                # slices + the four hidden pieces instead of the full 6MB.
                wpm = {}
                h0 = []
                for piece in range(NKP):
                    wx = pw1.tile(
                        [128, KH, 128], BF, tag=f"w{piece}_0", name=f"w1_{piece}_0"
                    )
                    nc.sync.dma_start(out=wx, in_=wT[0, :, ts(piece, KH), :])
                    wpm[(piece, 0)] = wx
                    hx = ph.tile(
                        [128, KH, 512], BF, tag=f"hid{piece}", name=f"h0_{piece}"
                    )
                    nc.sync.dma_start(out=hx, in_=hT[0, :, ts(piece, KH), :])
                    h0.append(hx)
                for m in range(1, MQKV):
                    for piece in range(NKP):
                        wx = pw1.tile(
                            [128, KH, 128],
                            BF,
                            tag=f"w{piece}_{m}",
                            name=f"w1_{piece}_{m}",
                        )
                        nc.sync.dma_start(
                            out=wx, in_=wT[m, :, ts(piece, KH), :]
                        )
                        wpm[(piece, m)] = wx
                # keep the PE busy on constants while the first weight/hidden
                # pieces land, so the clock ramp finishes before real work
                warm = ps_qkv.tile([128, 128], F32, tag="pt", name="warm")
                for i in range(48):
                    nc.tensor.matmul(
                        warm, lhsT=ones_mat, rhs=ones_mat,
                        start=(i == 0), stop=(i == 47),
                    )
                nc.sync.dma_start(out=swapmat, in_=swapmat_d[:, :])
                nc.sync.dma_start(out=tabAq, in_=ropeAq[:, :])
                nc.sync.dma_start(out=tabBq, in_=ropeBq[:, :])
                nc.sync.dma_start(out=tabAk, in_=ropeAk[:, :])
                nc.sync.dma_start(out=tabBk, in_=ropeBk[:, :])
                nc.sync.dma_start(out=mask_sb, in_=maskT[:, :, :])

                # rmsnorm + rope for one row-block x one chunk. The
                # sum-of-squares runs on the PE with an all-ones [128,128]
                # lhsT, which lands the result already broadcast across all
                # partitions (same PE cost as a 1-row reduce), so the
                # reciprocal runs full-channel and no broadcast matmul is
                # needed. rscale (the score scale) and eps fold into the Sqrt
                # activation's scale/bias: r = rscale/sqrt(ssq/D + eps)
                #   = 1/sqrt(ssq/(D*rscale^2) + eps/rscale^2).
                x2s = {}

                def norm_x2(m, ch):
                    # squares are emitted eagerly at the end of chunk ch's own
                    # m-loop so the DVE runs ahead of the lazily-popped ssq
                    # matmuls below
                    sl = ts(ch, 512)
                    src = qkvT_sb[:, m, sl]
                    x2 = paux.tile([128, 512], BF, tag="x2", bufs=6, name=f"x2_{m}_{ch}")
                    nc.vector.tensor_mul(x2, src, src)
                    x2s[(m, ch)] = x2

                def norm_chunk(m, ch, tabA, tabB, scl, bias_ap, out_ap):
                    sl = ts(ch, 512)
                    src = qkvT_sb[:, m, sl]
                    x2 = x2s.pop((m, ch))
                    ssq = ps_na.tile([128, 512], F32, tag="ssq", name=f"ssq_{m}_{ch}")
                    nc.tensor.matmul(ssq, lhsT=ones_mat, rhs=x2, start=True, stop=True)
                    # r = (ssq*scl + eps')^(-1/2) as exp(-0.5*ln(.)) — two
                    # ScalarE table ops, no sqrt and no (slow) DVE reciprocal
                    lg = paux.tile([128, 512], F32, tag="lg")
                    nc.scalar.activation(
                        lg,
                        ssq,
                        mybir.ActivationFunctionType.Ln,
                        scale=scl,
                        bias=bias_ap,
                    )
                    r = paux.tile([128, 512], F32, tag="r")
                    nc.scalar.activation(
                        r, lg, mybir.ActivationFunctionType.Exp, scale=-0.5
                    )
                    sw = ps_na.tile([128, 512], F32, tag="sw", bufs=3, name=f"sw_{m}_{ch}")
                    nc.tensor.matmul(sw, lhsT=swapmat, rhs=src, start=True, stop=True)
                    y = paux.tile([128, 512], BF, tag="y")
                    nc.vector.tensor_mul(y, src, tabA[:, sl])
                    tmp = paux.tile([128, 512], BF, tag="rtmp")
                    nc.vector.tensor_mul(tmp, sw, tabB[:, sl])
                    nc.vector.tensor_add(y, y, tmp)
                    nc.vector.tensor_mul(out_ap[:, sl], y, r)

                qscl = 1.0 / (D * (1.0 / D))  # rscale = 1/sqrt(D)

                def norm_op(m, ch, tabA, tabB, scl, bias_ap, out_ap):
                    def go():
                        with nc.named_scope("norm"):
                            norm_chunk(m, ch, tabA, tabB, scl, bias_ap, out_ap)

                    return go

                def vtrans_op(kt):
                    def go():
                        with nc.named_scope("vtrans"):
                            ptr = ps_na.tile(
                                [128, 128], BF, tag="vt", bufs=1, name=f"tr_{kt}"
                            )
                            nc.tensor.transpose(
                                ptr, qkvT_sb[:, 5, ts(kt, 128)], identity
                            )
                            nc.scalar.copy(out=v_sb[:, kt, :], in_=ptr)

                    return go

                # norm/vtrans work for chunk ch is spread through chunk ch+1's
                # m-loop (two ops per m-tile) so the PE's tiny norm matmuls
                # never wait on the DVE norm chains at a chunk boundary.
                lazy = []
                for nch in range(NCH):
                    if nch > 0:
                        h0 = []
                        for piece in range(NKP):
                            hx = ph.tile(
                                [128, KH, 512],
                                BF,
                                tag=f"hid{piece}",
                                name=f"h_{nch}_{piece}",
                            )
                            nc.sync.dma_start(
                                out=hx, in_=hT[nch, :, ts(piece, KH), :]
                            )
                            h0.append(hx)
                    def queue_norm(m):
                        if m == 4:
                            lazy.append(
                                norm_op(4, nch, tabAk, tabBk, 1.0 / D, bias_k, ks_sb)
                            )
                        else:
                            lazy.append(
                                norm_op(m, nch, tabAq, tabBq, qscl, bias_q, qs_sb[:, m])
                            )

                    # In the last chunk the k/q rows are computed first and
                    # their norms queued immediately, so the norm chains drain
                    # inside the m-loop instead of in a bubble before attn.
                    last = nch == NCH - 1
                    m_iter = (4, 0, 1, 2, 3, 5) if last else range(MQKV)
                    with nc.named_scope("qkv"):
                        for m in m_iter:
                            pt = ps_qkv.tile([128, 512], F32, tag="pt")
                            for k in range(KO):
                                nc.tensor.matmul(
                                    pt,
                                    lhsT=wpm[(k // KH, m)][:, k % KH, :],
                                    rhs=h0[k // KH][:, k % KH, :],
                                    start=(k == 0),
                                    stop=(k == KO - 1),
                                )
                            nc.scalar.copy(out=qkvT_sb[:, m, ts(nch, 512)], in_=pt)
                            if last and m != 5:
                                with nc.named_scope("norm"):
                                    norm_x2(m, nch)
                                queue_norm(m)
                            for _ in range(2):
                                if lazy:
                                    lazy.pop(0)()
                    if not last:
                        with nc.named_scope("norm"):
                            for m in (4, 0, 1, 2, 3):
                                norm_x2(m, nch)
                    # v transposes depend on nothing but the m=5 copies: emit
                    # them eagerly so the end-of-phase drain is norm-only
                    for kt in range(4 * nch, 4 * nch + 4):
                        vtrans_op(kt)()
                    if not last:
                        for m in (4, 0, 1, 2, 3):
                            queue_norm(m)
                while lazy:
                    lazy.pop(0)()

            # ---- phase B: attention + o_proj -------------------------------
            with (
                tc.tile_pool(name="ps_st", bufs=2, space="PSUM") as ps_st,
                tc.tile_pool(name="ps_at", bufs=2, space="PSUM") as ps_at,
                tc.tile_pool(name="ps_c", bufs=2, space="PSUM") as ps_c,
                tc.tile_pool(name="ppt", bufs=2) as ppt,
                tc.tile_pool(name="pdn", bufs=2) as pdn,
                tc.tile_pool(name="wo", bufs=1) as pw2,
                tc.tile_pool(name="outs", bufs=4) as pout,
            ):
                w2 = pw2.tile([128, HPC, HIDDEN], BF)
                nc.sync.dma_start(
                    out=w2, in_=w_oT.rearrange("(kk p) j -> p kk j", p=128)
                )

                # attention, normalize pipelined one (h,qc) behind. o_proj
                # matmuls of the previous q-chunk are interleaved into the
                # k-tile loops (from the second head block on, once every head
                # of the previous chunk is flushed) so the PE stays fed while
                # ScalarE streams exps.
                pending = None

                def flush(p):
                    # dn = colsum of the DVE-accumulated prob sum (one PE
                    # matmul instead of one per k-tile), then 1/dn as
                    # exp(-ln(dn)) on ScalarE — no slow DVE reciprocal
                    at_ps, accb, hh, qc = p
                    # dn borrows a slot in the st pool (its lifetime is the
                    # few instructions of this flush)
                    dn_ps = ps_st.tile([128, 512], F32, tag="st", name="dn")
                    nc.tensor.matmul(
                        dn_ps, lhsT=ones_mat, rhs=accb, start=True, stop=True
                    )
                    lgd = paux.tile([128, 512], F32, tag="lgd")
                    nc.scalar.activation(lgd, dn_ps, mybir.ActivationFunctionType.Ln)
                    rcp = paux.tile([128, 512], F32, tag="rcp")
                    nc.scalar.activation(
                        rcp, lgd, mybir.ActivationFunctionType.Exp, scale=-1.0
                    )
                    nc.vector.tensor_mul(attnT_sb[:, hh, ts(qc, 512)], at_ps, rcp)

                def o_proj_ops(qc):
                    # each entry is (kk, emit): kk gates popping in the first
                    # head block, where the previous chunk's head 3 is not yet
                    # flushed. Output tiles are [128, 2, 512] PSUM pairs (two
                    # banks) so each needs only one copy + one 256KB DMA.
                    ops = []
                    for mt in range(4 * qc, 4 * qc + 4):
                        for n2 in range(4):
                            state = {}
                            for kk in range(HPC):
                                for half in range(2):

                                    def op(mt=mt, n2=n2, kk=kk, half=half, state=state):
                                        with nc.named_scope("oproj"):
                                            if kk == 0 and half == 0:
                                                state["po"] = ps_c.tile(
                                                    [128, 2, 512],
                                                    F32,
                                                    tag="c",
                                                    name=f"po_{mt}_{n2}",
                                                )
                                            nc.tensor.matmul(
                                                state["po"][:, half, :],
                                                lhsT=attnT_sb[:, kk, ts(mt, 128)],
                                                rhs=w2[:, kk, ts(2 * n2 + half, 512)],
                                                start=(kk == 0),
                                                stop=(kk == HPC - 1),
                                            )
                                            if kk == HPC - 1 and half == 1:
                                                ot = pout.tile(
                                                    [128, 2, 512], BF, tag="ot"
                                                )
                                                # alternate the PSUM->SBUF
                                                # copies across DVE and ScalarE
                                                if (mt + n2) % 2 == 0:
                                                    nc.vector.tensor_copy(
                                                        ot, state["po"]
                                                    )
                                                else:
                                                    nc.scalar.copy(
                                                        out=ot, in_=state["po"]
                                                    )
                                                nc.sync.dma_start(
                                                    out=outp[
                                                        ts(mt, 128), ts(n2, 1024)
                                                    ],
                                                    in_=ot,
                                                )

                                    ops.append((kk, op))
                    return ops

                oproj = []
                for qc in range(NCH):
                    with nc.named_scope("attn"):
                        for hh in range(HPC):
                            nkt = 4 * (qc + 1)
                            ptile = ppt.tile([128, NKT, 512], BF, tag="pt")
                            at_ps = ps_at.tile([128, 512], F32, tag="at", name="at")
                            acc = pdn.tile([128, 512], F32, tag="acc", name="acc")
                            accb = pdn.tile([128, 512], BF, tag="accb", name="accb")

                            # Diagonal k-tiles only reach q-columns >= 128*r
                            # (r = kt - 4*qc): the score matmul, exp, mask,
                            # at-accumulation and dn adds all run on that
                            # shrinking subrange (512/384/256/128 cols), and
                            # the mask shrinks to the [128,128] triangle slice
                            # of the existing table.
                            def diag_off(kt):
                                r = kt - 4 * qc
                                return 128 * r if r >= 0 else 0

                            def st_exp(kt):
                                off = diag_off(kt)
                                st = ps_st.tile([128, 512], F32, tag="st", name="st")
                                nc.tensor.matmul(
                                    st[:, off:],
                                    lhsT=ks_sb[:, ts(kt, 128)],
                                    rhs=qs_sb[:, hh, 512 * qc + off : 512 * (qc + 1)],
                                    start=True,
                                    stop=True,
                                )
                                nc.scalar.activation(
                                    ptile[:, kt, off:],
                                    st[:, off:],
                                    mybir.ActivationFunctionType.Exp,
                                )
                                if kt >= 4 * qc:
                                    nc.vector.tensor_mul(
                                        ptile[:, kt, off : off + 128],
                                        ptile[:, kt, off : off + 128],
                                        mask_sb[:, kt - 4 * qc, off : off + 128],
                                    )

                            def at_mm(kt):
                                off = diag_off(kt)
                                nc.tensor.matmul(
                                    at_ps[:, off:],
                                    lhsT=v_sb[:, kt, :],
                                    rhs=ptile[:, kt, off:],
                                    start=(kt == 0),
                                    stop=(kt == nkt - 1),
                                    skip_group_check=True,
                                )

                            # prob sums for the softmax denominator accumulate
                            # on the DVE (in-place, subrange for diagonal
                            # tiles), cast to bf16 for flush's ones-matmul
                            def dn_acc(kt):
                                off = diag_off(kt)
                                if kt == 0:
                                    nc.vector.tensor_copy(acc, ptile[:, 0, :])
                                else:
                                    nc.vector.tensor_add(
                                        acc[:, off:], acc[:, off:], ptile[:, kt, off:]
                                    )
                                if kt == nkt - 1:
                                    nc.vector.tensor_copy(accb, acc)

                            # PE order: st(kt+1) is emitted before at(kt) so
                            # the PE never sits behind a matmul whose rhs is
                            # still being exp'd by ScalarE. o_proj pops are
                            # gated to hh >= 1: the last head of the previous
                            # chunk is flushed at the end of block (0, qc).
                            st_exp(0)
                            for kt in range(1, nkt):
                                st_exp(kt)
                                at_mm(kt - 1)
                                dn_acc(kt - 1)
                                if oproj and (hh >= 1 or oproj[0][0] < HPC - 1):
                                    oproj.pop(0)[1]()
                            at_mm(nkt - 1)
                            dn_acc(nkt - 1)
                            if pending is not None:
                                flush(pending)
                            pending = (at_ps, accb, hh, qc)
                    with nc.named_scope("oproj"):
                        while oproj:
                            oproj.pop(0)[1]()
                    oproj = o_proj_ops(qc)
                with nc.named_scope("attn"):
                    flush(pending)
                with nc.named_scope("oproj"):
                    while oproj:
                        oproj.pop(0)[1]()

    _split_waits(nc)
    return nc


_MAX_WAITS = 1


def _split_waits(nc, max_waits=_MAX_WAITS):
    """This walrus build rejects instructions carrying more than one sync-wait
    ("Too many sync wait commands"). Peel excess waits onto NOPs emitted just
    before the instruction on the same engine (same-engine waits execute in
    program order, so semantics are unchanged)."""
    n_split = 0
    for f in nc.m.functions:
        for b in f.blocks:
            out = []
            for ins in b.instructions:
                si = getattr(ins, "sync_info", None)
                ow = list(si.on_wait) if si is not None and si.on_wait else []
                if len(ow) > max_waits:
                    keep = ow[-max_waits:]
                    excess = ow[: -max_waits]
                    for i in range(0, len(excess), max_waits):
                        chunk = excess[i : i + max_waits]
                        out.append(
                            mybir.InstNoOp(
                                name=f"{ins.name}-wait{i}",
                                engine=ins.engine,
                                sync_info=mybir.SyncInfo(on_wait=chunk, on_update=[]),
                            )
                        )
                    ins.sync_info = mybir.SyncInfo(
                        on_wait=keep, on_update=list(si.on_update or [])
                    )
                    n_split += 1
                out.append(ins)
            b.instructions = out
    return n_split


_NC = None


def _get_nc():
    global _NC
    if _NC is None:
        _NC = build_nc()
    return _NC


def _host_inputs(hidden_states, positions, w_qkv, w_o, q_norm_w, k_norm_w):
    """Build the 8 per-core input maps (numpy, bf16 where matmul operands)."""
    hiddenT = np.ascontiguousarray(hidden_states.astype(np.float32).T).astype(BF16)
    # chunk-major: [NCH, HIDDEN, 512] so each (chunk, k-piece) DMA is one
    # contiguous DRAM region
    hiddenT = np.ascontiguousarray(
        hiddenT.reshape(HIDDEN, T // 512, 512).transpose(1, 0, 2)
    )

    pos = np.asarray(positions).astype(np.float64)
    half = D // 2
    inv_freq = 1.0 / (ROPE_THETA ** (np.arange(half, dtype=np.float64) / half))
    freqs = pos[:, None] * inv_freq  # [T, 64]
    cos = np.cos(freqs).T  # [64, T]
    sin = np.sin(freqs).T

    def tables(w):
        w = np.asarray(w, dtype=np.float64)
        w1 = w[:half][:, None]
        w2 = w[half:][:, None]
        A = np.concatenate([cos * w1, cos * w2], axis=0)
        B = np.concatenate([-sin * w2, sin * w1], axis=0)
        return A.astype(BF16), B.astype(BF16)

    Aq, Bq = tables(q_norm_w)
    Ak, Bk = tables(k_norm_w)

    dk = np.arange(128)[:, None, None]
    rr = np.arange(4)[None, :, None]
    dq = np.arange(512)[None, None, :]
    mask = (128 * rr + dk <= dq).astype(BF16)  # [128, 4, 512]
    swap = np.roll(np.eye(128, dtype=np.float32), 64, axis=1).astype(BF16)

    q_size = 32 * D  # 4096
    kv_size = 8 * D  # 1024
    in_maps = []
    for c in range(N_CORES):
        qrows = w_qkv[512 * c : 512 * (c + 1)]
        krows = w_qkv[q_size + D * c : q_size + D * (c + 1)]
        vrows = w_qkv[q_size + kv_size + D * c : q_size + kv_size + D * (c + 1)]
        wl = np.concatenate([qrows, krows, vrows], axis=0).astype(np.float32)
        w_qkvT_c = np.ascontiguousarray(wl.T).astype(BF16)  # [4096, 768]
        # m-tile-major: [6, 32, 128, 128] so (k-piece, m) slices are
        # contiguous in DRAM
        w_qkvT_c = np.ascontiguousarray(
            w_qkvT_c.reshape(32, 128, 6, 128).transpose(2, 0, 1, 3)
        )
        w_oT_c = np.ascontiguousarray(
            w_o[:, 512 * c : 512 * (c + 1)].astype(np.float32).T
        ).astype(BF16)  # [512, 4096]
        in_maps.append(
            {
                "hiddenT": hiddenT,
                "w_qkvT": w_qkvT_c,
                "w_oT": w_oT_c,
                "ropeAq": Aq,
                "ropeBq": Bq,
                "ropeAk": Ak,
                "ropeBk": Bk,
                "maskT": mask,
                "swapmat": swap,
            }
        )
    return in_maps


_LAST_PERF = {}


def kernel(hidden_states, positions, w_qkv, w_o, q_norm_w, k_norm_w):
    trace = os.environ.get("KERNEL_TRACE", "0") == "1"
    if trace:
        _enable_tracing()
    from concourse.bass_utils import run_bass_kernel_spmd

    nc = _get_nc()
    in_maps = _host_inputs(hidden_states, positions, w_qkv, w_o, q_norm_w, k_norm_w)
    res = run_bass_kernel_spmd(
        nc, in_maps, core_ids=list(range(N_CORES)), trace=trace
    )
    _LAST_PERF["exec_time_ns"] = res.exec_time_ns
    _LAST_PERF["trace"] = (
        res.instructions_and_trace[1] if res.instructions_and_trace else None
    )
    _LAST_PERF["insts"] = (
        res.instructions_and_trace[0] if res.instructions_and_trace else None
    )
    _LAST_PERF["scopes"] = res.per_core_scope_times
    out = np.zeros((T, HIDDEN), dtype=np.float64)
    for r in res.results:
        out += r["outp"].astype(np.float64)
    return out.astype(np.float32)



# revision 80
# speedup vs baseline: 1.0045x; 1.0045x over previous
"""HYV3Attention (qkv proj + qk-RMSNorm + neox RoPE + causal GQA attention +
o_proj) on 8 Trainium2 NeuronCores.

Sharding: tensor-parallel across heads. Core c owns q heads 4c..4c+3 and kv
head c (GQA group c), i.e. 768 of the 6144 qkv_proj rows and 512 of the 4096
o_proj columns. Each core produces a full [T, HIDDEN] partial of the output
(o_proj contracts only over its own heads); the host sums the 8 partials.
No collectives.

Per-core device kernel (all matmuls bf16, f32 accumulation), tuned so the PE
issues back-to-back (measured ~216ns per 512-col matmul at full clock):
  1. qkvT = w_local @ hidden.T -> [768, 2048] "feature-on-partition", weights
     DMA'd in per-(k-piece, m-tile) 256KB slices so the first matmul starts
     after <1MB has landed.
  2. RMSNorm via sum-of-squares matmul with an all-ones [128,128] lhsT (lands
     the reduce already broadcast across partitions; same PE cost as a 1-row
     reduce); r = (ssq*scl+eps')^-1/2 computed as exp(-0.5*ln(.)) — two
     ScalarE table ops, no sqrt and no (slow, ~6cyc/elem) DVE reciprocal.
     RoPE as elementwise multiplies against host-precomputed cos/sin tables
     with the norm weights folded in (the half-rotation comes from a PE
     permutation matmul); 1/sqrt(HEAD_DIM) and eps fold into ln's scale/bias.
     Each chunk's norm work is spread through the NEXT chunk's m-loop (and the
     last chunk reorders its m-tiles k-first) so DVE/ScalarE latency hides
     under the qkv matmul stream.
  3. Scores S.T tile [k=128, q=512] = kT.T @ qT ; softmax without max
     subtraction (RMS-normed scores are bounded by sqrt(128)); exp on ScalarE
     straight out of PSUM; causal masking by multiplying the 4 diagonal
     k-tiles with 0/1 masks; attnT accumulates in PSUM with lhsT = v-tiles;
     softmax denominators accumulate Σ_kt p on the DVE (in-place adds) with a
     single all-ones matmul + exp(-ln(dn)) + multiply at flush time.
  4. out_partial = attn_flat @ w_o_slice.T with lhsT = attnT tiles (attnT
     aliases the dead q rows of qkvT). o_proj matmuls of the previous q-chunk
     are interleaved into the exp-gated k-tile loops to fill PE bubbles;
     outputs accumulate in [128, 2, 512] two-bank PSUM pairs (one copy + one
     256KB DMA each, alternating DVE/ScalarE), written bf16 and summed f32 on
     the host.
"""
import os

import numpy as np
import ml_dtypes

import concourse.bass as bass
import concourse.mybir as mybir
import concourse.tile as tile
from concourse.bass import ts
from concourse.masks import make_identity

BF16 = ml_dtypes.bfloat16
F32 = mybir.dt.float32
BF = mybir.dt.bfloat16

T = 2048
HIDDEN = 4096
D = 128  # head dim
N_CORES = 8
HPC = 4  # q heads per core
KO = HIDDEN // 128  # 32 contraction tiles for qkv proj
MQKV = (HPC + 2) * D // 128  # 6 partition tiles of qkvT (4 q heads, k, v)
NCH = T // 512  # 4 free-dim chunks of 512
NKT = T // 128  # 16 k tiles
ROPE_THETA = 10000.0
RMS_EPS = 1e-5

# ---------------------------------------------------------------------------
# Workaround: this walrus build rejects Drain instructions carrying more than
# one sem-wait ("Too many sync wait commands"). Split the Tile tail drain into
# one Drain per outstanding logical proc, each with a single wait.
_PATCHED = False


def _patch_tile_tail():
    global _PATCHED
    if _PATCHED:
        return
    _PATCHED = True
    import concourse.tile as ctile
    from concourse.vector_clock import ScopedClock, VectorClock

    def _drain_and_barrier_split(self, tick_clock, wait_clock):
        gc = tick_clock.global_clock
        n = len(gc)
        for p in range(n):
            if gc[p] == 0:
                continue
            partial = VectorClock([gc[i] if i == p else 0 for i in range(n)])
            d = self.nc.sync.drain()
            wait_clock.add_sem_waits(d.ins, ScopedClock({None: partial}))
        self.nc.all_engine_barrier()
        assert self.sems is not None
        popped = self.nc._tile_sem_poison_stack.pop()
        assert popped is self._sem_poison
        self.nc.clear_and_free_semaphores(list(self.sems.allocated().values()))
        self.nc.all_engine_barrier()

    ctile.TileContext._drain_and_barrier = _drain_and_barrier_split


# ---------------------------------------------------------------------------
# Optional NTFF tracing support (KERNEL_TRACE=1): register the axon profile
# hook that this image's antenv lacks, and neuter the S3 artifact upload.
def _enable_tracing():
    import sys
    import types

    if "antenv.axon_hooks" not in sys.modules:
        holder = {"hook": None}
        mod = types.ModuleType("antenv.axon_hooks")
        mod.set_axon_ntff_profile_hook = lambda h: holder.__setitem__("hook", h)
        mod.get_axon_ntff_profile_hook = lambda: holder["hook"]
        sys.modules["antenv.axon_hooks"] = mod
        from trn_agent_boot.trn_boot import _ntff_profile_via_ctypes

        mod.set_axon_ntff_profile_hook(
            _ntff_profile_via_ctypes("/opt/axon/libaxon_pjrt.so")
        )
    import concourse.bass_utils as bu

    bu.upload_artifacts = lambda tmpdir: f"file://{tmpdir}"


# ---------------------------------------------------------------------------
def build_nc():
    _patch_tile_tail()
    nc = bass.Bass()

    # hiddenT is staged chunk-major ([chunk, hidden, 512]) so each (chunk,
    # k-piece) DMA reads one contiguous DRAM region (aggregatable packets)
    hiddenT = nc.dram_tensor("hiddenT", [NCH, HIDDEN, 512], BF, kind="ExternalInput")
    w_qkvT = nc.dram_tensor("w_qkvT", [HIDDEN, MQKV * 128], BF, kind="ExternalInput")
    w_oT = nc.dram_tensor("w_oT", [HPC * D, HIDDEN], BF, kind="ExternalInput")
    ropeAq = nc.dram_tensor("ropeAq", [D, T], BF, kind="ExternalInput")
    ropeBq = nc.dram_tensor("ropeBq", [D, T], BF, kind="ExternalInput")
    ropeAk = nc.dram_tensor("ropeAk", [D, T], BF, kind="ExternalInput")
    ropeBk = nc.dram_tensor("ropeBk", [D, T], BF, kind="ExternalInput")
    maskT = nc.dram_tensor("maskT", [128, 4, 512], BF, kind="ExternalInput")
    swapmat_d = nc.dram_tensor("swapmat", [128, 128], BF, kind="ExternalInput")
    outp = nc.dram_tensor("outp", [T, HIDDEN], BF, kind="ExternalOutput")

    with tile.TileContext(nc) as tc:
        with (
            tc.tile_pool(name="const", bufs=1) as pconst,
            tc.tile_pool(name="qkv", bufs=1) as pqkv,
            tc.tile_pool(name="aux", bufs=2) as paux,
            tc.tile_pool(name="qk_rope", bufs=1) as pqk,
        ):
            # ---- constants -------------------------------------------------
            identity = pconst.tile([128, 128], BF)
            make_identity(nc, identity)
            ones_mat = pconst.tile([128, 128], BF)
            nc.vector.memset(ones_mat, 1.0)
            # RMS eps with the q/k score scales folded in (see norm_chunk)
            bias_k = pconst.tile([128, 1], F32)
            nc.vector.memset(bias_k, RMS_EPS)
            bias_q = pconst.tile([128, 1], F32)
            nc.vector.memset(bias_q, RMS_EPS * D)
            swapmat = pconst.tile([128, 128], BF)

            qkvT_sb = pqkv.tile([128, MQKV, T], BF)
            # attnT aliases the q-head rows of qkvT: every norm read of a
            # (hh, chunk) slice completes before that slice's flush write
            # (flush depends on qs which depends on the same norm reads), and
            # disjoint slices carry no dependency. Saves 2MB of SBUF.
            attnT_sb = qkvT_sb[:, 0:HPC, :]

            tabAq = pqk.tile([D, T], BF)
            tabBq = pqk.tile([D, T], BF)
            tabAk = pqk.tile([D, T], BF)
            tabBk = pqk.tile([D, T], BF)
            mask_sb = pqk.tile([128, 4, 512], BF)
            qs_sb = pqk.tile([128, HPC, T], BF)  # roped+scaled q per head
            ks_sb = pqk.tile([128, T], BF)  # roped+scaled k
            v_sb = pqk.tile([128, NKT, D], BF)  # v in [token, d] layout

            # ---- phase A: qkvT = w_local @ hidden.T, with the norms, rope
            # and v transposes interleaved per 512-token chunk so the DVE /
            # ScalarE work hides under the qkv matmul stream ----------------
            with (
                tc.tile_pool(name="wq", bufs=1) as pw1,
                tc.tile_pool(name="hid", bufs=2) as ph,
                tc.tile_pool(name="ps_qkv", bufs=2, space="PSUM") as ps_qkv,
                tc.tile_pool(name="ps_na", bufs=2, space="PSUM") as ps_na,
            ):
                NKP = 4  # k-dim DMA pieces
                KH = KO // NKP
                wT = w_qkvT.rearrange("(ko p) m -> p ko m", p=128)
                hT = hiddenT.rearrange("c (ko p) t -> c p ko t", p=128)
                # The weight load is sliced per (k-piece, m-tile): the very
                # first matmul chain (m=0) only waits for the four 256KB m=0
                # slices + the four hidden pieces instead of the full 6MB.
                wpm = {}
                h0 = []
                for piece in range(NKP):
                    wx = pw1.tile(
                        [128, KH, 128], BF, tag=f"w{piece}_0", name=f"w1_{piece}_0"
                    )
                    nc.sync.dma_start(out=wx, in_=wT[:, ts(piece, KH), ts(0, 128)])
                    wpm[(piece, 0)] = wx
                    hx = ph.tile(
                        [128, KH, 512], BF, tag=f"hid{piece}", name=f"h0_{piece}"
                    )
                    nc.sync.dma_start(out=hx, in_=hT[0, :, ts(piece, KH), :])
                    h0.append(hx)
                for m in range(1, MQKV):
                    for piece in range(NKP):
                        wx = pw1.tile(
                            [128, KH, 128],
                            BF,
                            tag=f"w{piece}_{m}",
                            name=f"w1_{piece}_{m}",
                        )
                        nc.sync.dma_start(
                            out=wx, in_=wT[:, ts(piece, KH), ts(m, 128)]
                        )
                        wpm[(piece, m)] = wx
                # keep the PE busy on constants while the first weight/hidden
                # pieces land, so the clock ramp finishes before real work
                warm = ps_qkv.tile([128, 128], F32, tag="pt", name="warm")
                for i in range(48):
                    nc.tensor.matmul(
                        warm, lhsT=ones_mat, rhs=ones_mat,
                        start=(i == 0), stop=(i == 47),
                    )
                nc.sync.dma_start(out=swapmat, in_=swapmat_d[:, :])
                nc.sync.dma_start(out=tabAq, in_=ropeAq[:, :])
                nc.sync.dma_start(out=tabBq, in_=ropeBq[:, :])
                nc.sync.dma_start(out=tabAk, in_=ropeAk[:, :])
                nc.sync.dma_start(out=tabBk, in_=ropeBk[:, :])
                nc.sync.dma_start(out=mask_sb, in_=maskT[:, :, :])

                # rmsnorm + rope for one row-block x one chunk. The
                # sum-of-squares runs on the PE with an all-ones [128,128]
                # lhsT, which lands the result already broadcast across all
                # partitions (same PE cost as a 1-row reduce), so the
                # reciprocal runs full-channel and no broadcast matmul is
                # needed. rscale (the score scale) and eps fold into the Sqrt
                # activation's scale/bias: r = rscale/sqrt(ssq/D + eps)
                #   = 1/sqrt(ssq/(D*rscale^2) + eps/rscale^2).
                x2s = {}

                def norm_x2(m, ch):
                    # squares are emitted eagerly at the end of chunk ch's own
                    # m-loop so the DVE runs ahead of the lazily-popped ssq
                    # matmuls below
                    sl = ts(ch, 512)
                    src = qkvT_sb[:, m, sl]
                    x2 = paux.tile([128, 512], BF, tag="x2", bufs=6, name=f"x2_{m}_{ch}")
                    nc.vector.tensor_mul(x2, src, src)
                    x2s[(m, ch)] = x2

                def norm_chunk(m, ch, tabA, tabB, scl, bias_ap, out_ap):
                    sl = ts(ch, 512)
                    src = qkvT_sb[:, m, sl]
                    x2 = x2s.pop((m, ch))
                    ssq = ps_na.tile([128, 512], F32, tag="ssq", name=f"ssq_{m}_{ch}")
                    nc.tensor.matmul(ssq, lhsT=ones_mat, rhs=x2, start=True, stop=True)
                    # r = (ssq*scl + eps')^(-1/2) as exp(-0.5*ln(.)) — two
                    # ScalarE table ops, no sqrt and no (slow) DVE reciprocal
                    lg = paux.tile([128, 512], F32, tag="lg")
                    nc.scalar.activation(
                        lg,
                        ssq,
                        mybir.ActivationFunctionType.Ln,
                        scale=scl,
                        bias=bias_ap,
                    )
                    r = paux.tile([128, 512], F32, tag="r")
                    nc.scalar.activation(
                        r, lg, mybir.ActivationFunctionType.Exp, scale=-0.5
                    )
                    sw = ps_na.tile([128, 512], F32, tag="sw", bufs=3, name=f"sw_{m}_{ch}")
                    nc.tensor.matmul(sw, lhsT=swapmat, rhs=src, start=True, stop=True)
                    y = paux.tile([128, 512], BF, tag="y")
                    nc.vector.tensor_mul(y, src, tabA[:, sl])
                    tmp = paux.tile([128, 512], BF, tag="rtmp")
                    nc.vector.tensor_mul(tmp, sw, tabB[:, sl])
                    nc.vector.tensor_add(y, y, tmp)
                    nc.vector.tensor_mul(out_ap[:, sl], y, r)

                qscl = 1.0 / (D * (1.0 / D))  # rscale = 1/sqrt(D)

                def norm_op(m, ch, tabA, tabB, scl, bias_ap, out_ap):
                    def go():
                        with nc.named_scope("norm"):
                            norm_chunk(m, ch, tabA, tabB, scl, bias_ap, out_ap)

                    return go

                def vtrans_op(kt):
                    def go():
                        with nc.named_scope("vtrans"):
                            ptr = ps_na.tile(
                                [128, 128], BF, tag="vt", bufs=1, name=f"tr_{kt}"
                            )
                            nc.tensor.transpose(
                                ptr, qkvT_sb[:, 5, ts(kt, 128)], identity
                            )
                            nc.scalar.copy(out=v_sb[:, kt, :], in_=ptr)

                    return go

                # norm/vtrans work for chunk ch is spread through chunk ch+1's
                # m-loop (two ops per m-tile) so the PE's tiny norm matmuls
                # never wait on the DVE norm chains at a chunk boundary.
                lazy = []
                for nch in range(NCH):
                    if nch > 0:
                        h0 = []
                        for piece in range(NKP):
                            hx = ph.tile(
                                [128, KH, 512],
                                BF,
                                tag=f"hid{piece}",
                                name=f"h_{nch}_{piece}",
                            )
                            nc.sync.dma_start(
                                out=hx, in_=hT[nch, :, ts(piece, KH), :]
                            )
                            h0.append(hx)
                    def queue_norm(m):
                        if m == 4:
                            lazy.append(
                                norm_op(4, nch, tabAk, tabBk, 1.0 / D, bias_k, ks_sb)
                            )
                        else:
                            lazy.append(
                                norm_op(m, nch, tabAq, tabBq, qscl, bias_q, qs_sb[:, m])
                            )

                    # In the last chunk the k/q rows are computed first and
                    # their norms queued immediately, so the norm chains drain
                    # inside the m-loop instead of in a bubble before attn.
                    last = nch == NCH - 1
                    m_iter = (4, 0, 1, 2, 3, 5) if last else range(MQKV)
                    with nc.named_scope("qkv"):
                        for m in m_iter:
                            pt = ps_qkv.tile([128, 512], F32, tag="pt")
                            for k in range(KO):
                                nc.tensor.matmul(
                                    pt,
                                    lhsT=wpm[(k // KH, m)][:, k % KH, :],
                                    rhs=h0[k // KH][:, k % KH, :],
                                    start=(k == 0),
                                    stop=(k == KO - 1),
                                )
                            nc.scalar.copy(out=qkvT_sb[:, m, ts(nch, 512)], in_=pt)
                            if last and m != 5:
                                with nc.named_scope("norm"):
                                    norm_x2(m, nch)
                                queue_norm(m)
                            for _ in range(2):
                                if lazy:
                                    lazy.pop(0)()
                    if not last:
                        with nc.named_scope("norm"):
                            for m in (4, 0, 1, 2, 3):
                                norm_x2(m, nch)
                    # v transposes depend on nothing but the m=5 copies: emit
                    # them eagerly so the end-of-phase drain is norm-only
                    for kt in range(4 * nch, 4 * nch + 4):
                        vtrans_op(kt)()
                    if not last:
                        for m in (4, 0, 1, 2, 3):
                            queue_norm(m)
                while lazy:
                    lazy.pop(0)()

            # ---- phase B: attention + o_proj -------------------------------
            with (
                tc.tile_pool(name="ps_st", bufs=2, space="PSUM") as ps_st,
                tc.tile_pool(name="ps_at", bufs=2, space="PSUM") as ps_at,
                tc.tile_pool(name="ps_c", bufs=2, space="PSUM") as ps_c,
                tc.tile_pool(name="ppt", bufs=2) as ppt,
                tc.tile_pool(name="pdn", bufs=2) as pdn,
                tc.tile_pool(name="wo", bufs=1) as pw2,
                tc.tile_pool(name="outs", bufs=4) as pout,
            ):
                w2 = pw2.tile([128, HPC, HIDDEN], BF)
                nc.sync.dma_start(
                    out=w2, in_=w_oT.rearrange("(kk p) j -> p kk j", p=128)
                )

                # attention, normalize pipelined one (h,qc) behind. o_proj
                # matmuls of the previous q-chunk are interleaved into the
                # k-tile loops (from the second head block on, once every head
                # of the previous chunk is flushed) so the PE stays fed while
                # ScalarE streams exps.
                pending = None

                def flush(p):
                    # dn = colsum of the DVE-accumulated prob sum (one PE
                    # matmul instead of one per k-tile), then 1/dn as
                    # exp(-ln(dn)) on ScalarE — no slow DVE reciprocal
                    at_ps, accb, hh, qc = p
                    # dn borrows a slot in the st pool (its lifetime is the
                    # few instructions of this flush)
                    dn_ps = ps_st.tile([128, 512], F32, tag="st", name="dn")
                    nc.tensor.matmul(
                        dn_ps, lhsT=ones_mat, rhs=accb, start=True, stop=True
                    )
                    lgd = paux.tile([128, 512], F32, tag="lgd")
                    nc.scalar.activation(lgd, dn_ps, mybir.ActivationFunctionType.Ln)
                    rcp = paux.tile([128, 512], F32, tag="rcp")
                    nc.scalar.activation(
                        rcp, lgd, mybir.ActivationFunctionType.Exp, scale=-1.0
                    )
                    nc.vector.tensor_mul(attnT_sb[:, hh, ts(qc, 512)], at_ps, rcp)

                def o_proj_ops(qc):
                    # each entry is (kk, emit): kk gates popping in the first
                    # head block, where the previous chunk's head 3 is not yet
                    # flushed. Output tiles are [128, 2, 512] PSUM pairs (two
                    # banks) so each needs only one copy + one 256KB DMA.
                    ops = []
                    for mt in range(4 * qc, 4 * qc + 4):
                        for n2 in range(4):
                            state = {}
                            for kk in range(HPC):
                                for half in range(2):

                                    def op(mt=mt, n2=n2, kk=kk, half=half, state=state):
                                        with nc.named_scope("oproj"):
                                            if kk == 0 and half == 0:
                                                state["po"] = ps_c.tile(
                                                    [128, 2, 512],
                                                    F32,
                                                    tag="c",
                                                    name=f"po_{mt}_{n2}",
                                                )
                                            nc.tensor.matmul(
                                                state["po"][:, half, :],
                                                lhsT=attnT_sb[:, kk, ts(mt, 128)],
                                                rhs=w2[:, kk, ts(2 * n2 + half, 512)],
                                                start=(kk == 0),
                                                stop=(kk == HPC - 1),
                                            )
                                            if kk == HPC - 1 and half == 1:
                                                ot = pout.tile(
                                                    [128, 2, 512], BF, tag="ot"
                                                )
                                                # alternate the PSUM->SBUF
                                                # copies across DVE and ScalarE
                                                if (mt + n2) % 2 == 0:
                                                    nc.vector.tensor_copy(
                                                        ot, state["po"]
                                                    )
                                                else:
                                                    nc.scalar.copy(
                                                        out=ot, in_=state["po"]
                                                    )
                                                nc.sync.dma_start(
                                                    out=outp[
                                                        ts(mt, 128), ts(n2, 1024)
                                                    ],
                                                    in_=ot,
                                                )

                                    ops.append((kk, op))
                    return ops

                oproj = []
                for qc in range(NCH):
                    with nc.named_scope("attn"):
                        for hh in range(HPC):
                            nkt = 4 * (qc + 1)
                            ptile = ppt.tile([128, NKT, 512], BF, tag="pt")
                            at_ps = ps_at.tile([128, 512], F32, tag="at", name="at")
                            acc = pdn.tile([128, 512], F32, tag="acc", name="acc")
                            accb = pdn.tile([128, 512], BF, tag="accb", name="accb")

                            # Diagonal k-tiles only reach q-columns >= 128*r
                            # (r = kt - 4*qc): the score matmul, exp, mask,
                            # at-accumulation and dn adds all run on that
                            # shrinking subrange (512/384/256/128 cols), and
                            # the mask shrinks to the [128,128] triangle slice
                            # of the existing table.
                            def diag_off(kt):
                                r = kt - 4 * qc
                                return 128 * r if r >= 0 else 0

                            def st_exp(kt):
                                off = diag_off(kt)
                                st = ps_st.tile([128, 512], F32, tag="st", name="st")
                                nc.tensor.matmul(
                                    st[:, off:],
                                    lhsT=ks_sb[:, ts(kt, 128)],
                                    rhs=qs_sb[:, hh, 512 * qc + off : 512 * (qc + 1)],
                                    start=True,
                                    stop=True,
                                )
                                nc.scalar.activation(
                                    ptile[:, kt, off:],
                                    st[:, off:],
                                    mybir.ActivationFunctionType.Exp,
                                )
                                if kt >= 4 * qc:
                                    nc.vector.tensor_mul(
                                        ptile[:, kt, off : off + 128],
                                        ptile[:, kt, off : off + 128],
                                        mask_sb[:, kt - 4 * qc, off : off + 128],
                                    )

                            def at_mm(kt):
                                off = diag_off(kt)
                                nc.tensor.matmul(
                                    at_ps[:, off:],
                                    lhsT=v_sb[:, kt, :],
                                    rhs=ptile[:, kt, off:],
                                    start=(kt == 0),
                                    stop=(kt == nkt - 1),
                                    skip_group_check=True,
                                )

                            # prob sums for the softmax denominator accumulate
                            # on the DVE (in-place, subrange for diagonal
                            # tiles), cast to bf16 for flush's ones-matmul
                            def dn_acc(kt):
                                off = diag_off(kt)
                                if kt == 0:
                                    nc.vector.tensor_copy(acc, ptile[:, 0, :])
                                else:
                                    nc.vector.tensor_add(
                                        acc[:, off:], acc[:, off:], ptile[:, kt, off:]
                                    )
                                if kt == nkt - 1:
                                    nc.vector.tensor_copy(accb, acc)

                            # PE order: st(kt+1) is emitted before at(kt) so
                            # the PE never sits behind a matmul whose rhs is
                            # still being exp'd by ScalarE. o_proj pops are
                            # gated to hh >= 1: the last head of the previous
                            # chunk is flushed at the end of block (0, qc).
                            st_exp(0)
                            for kt in range(1, nkt):
                                st_exp(kt)
                                at_mm(kt - 1)
                                dn_acc(kt - 1)
                                if oproj and (hh >= 1 or oproj[0][0] < HPC - 1):
                                    oproj.pop(0)[1]()
                            at_mm(nkt - 1)
                            dn_acc(nkt - 1)
                            if pending is not None:
                                flush(pending)
                            pending = (at_ps, accb, hh, qc)
                    with nc.named_scope("oproj"):
                        while oproj:
                            oproj.pop(0)[1]()
                    oproj = o_proj_ops(qc)
                with nc.named_scope("attn"):
                    flush(pending)
                with nc.named_scope("oproj"):
                    while oproj:
                        oproj.pop(0)[1]()

    _split_waits(nc)
    return nc


_MAX_WAITS = 1


def _split_waits(nc, max_waits=_MAX_WAITS):
    """This walrus build rejects instructions carrying more than one sync-wait
    ("Too many sync wait commands"). Peel excess waits onto NOPs emitted just
    before the instruction on the same engine (same-engine waits execute in
    program order, so semantics are unchanged)."""
    n_split = 0
    for f in nc.m.functions:
        for b in f.blocks:
            out = []
            for ins in b.instructions:
                si = getattr(ins, "sync_info", None)
                ow = list(si.on_wait) if si is not None and si.on_wait else []
                if len(ow) > max_waits:
                    keep = ow[-max_waits:]
                    excess = ow[: -max_waits]
                    for i in range(0, len(excess), max_waits):
                        chunk = excess[i : i + max_waits]
                        out.append(
                            mybir.InstNoOp(
                                name=f"{ins.name}-wait{i}",
                                engine=ins.engine,
                                sync_info=mybir.SyncInfo(on_wait=chunk, on_update=[]),
                            )
                        )
                    ins.sync_info = mybir.SyncInfo(
                        on_wait=keep, on_update=list(si.on_update or [])
                    )
                    n_split += 1
                out.append(ins)
            b.instructions = out
    return n_split


_NC = None


def _get_nc():
    global _NC
    if _NC is None:
        _NC = build_nc()
    return _NC


def _host_inputs(hidden_states, positions, w_qkv, w_o, q_norm_w, k_norm_w):
    """Build the 8 per-core input maps (numpy, bf16 where matmul operands)."""
    hiddenT = np.ascontiguousarray(hidden_states.astype(np.float32).T).astype(BF16)
    # chunk-major: [NCH, HIDDEN, 512] so each (chunk, k-piece) DMA is one
    # contiguous DRAM region
    hiddenT = np.ascontiguousarray(
        hiddenT.reshape(HIDDEN, T // 512, 512).transpose(1, 0, 2)
    )

    pos = np.asarray(positions).astype(np.float64)
    half = D // 2
    inv_freq = 1.0 / (ROPE_THETA ** (np.arange(half, dtype=np.float64) / half))
    freqs = pos[:, None] * inv_freq  # [T, 64]
    cos = np.cos(freqs).T  # [64, T]
    sin = np.sin(freqs).T

    def tables(w):
        w = np.asarray(w, dtype=np.float64)
        w1 = w[:half][:, None]
        w2 = w[half:][:, None]
        A = np.concatenate([cos * w1, cos * w2], axis=0)
        B = np.concatenate([-sin * w2, sin * w1], axis=0)
        return A.astype(BF16), B.astype(BF16)

    Aq, Bq = tables(q_norm_w)
    Ak, Bk = tables(k_norm_w)

    dk = np.arange(128)[:, None, None]
    rr = np.arange(4)[None, :, None]
    dq = np.arange(512)[None, None, :]
    mask = (128 * rr + dk <= dq).astype(BF16)  # [128, 4, 512]
    swap = np.roll(np.eye(128, dtype=np.float32), 64, axis=1).astype(BF16)

    q_size = 32 * D  # 4096
    kv_size = 8 * D  # 1024
    in_maps = []
    for c in range(N_CORES):
        qrows = w_qkv[512 * c : 512 * (c + 1)]
        krows = w_qkv[q_size + D * c : q_size + D * (c + 1)]
        vrows = w_qkv[q_size + kv_size + D * c : q_size + kv_size + D * (c + 1)]
        wl = np.concatenate([qrows, krows, vrows], axis=0).astype(np.float32)
        w_qkvT_c = np.ascontiguousarray(wl.T).astype(BF16)  # [4096, 768]
        w_oT_c = np.ascontiguousarray(
            w_o[:, 512 * c : 512 * (c + 1)].astype(np.float32).T
        ).astype(BF16)  # [512, 4096]
        in_maps.append(
            {
                "hiddenT": hiddenT,
                "w_qkvT": w_qkvT_c,
                "w_oT": w_oT_c,
                "ropeAq": Aq,
                "ropeBq": Bq,
                "ropeAk": Ak,
                "ropeBk": Bk,
                "maskT": mask,
                "swapmat": swap,
            }
        )
    return in_maps


_LAST_PERF = {}


def kernel(hidden_states, positions, w_qkv, w_o, q_norm_w, k_norm_w):
    trace = os.environ.get("KERNEL_TRACE", "0") == "1"
    if trace:
        _enable_tracing()
    from concourse.bass_utils import run_bass_kernel_spmd

    nc = _get_nc()
    in_maps = _host_inputs(hidden_states, positions, w_qkv, w_o, q_norm_w, k_norm_w)
    res = run_bass_kernel_spmd(
        nc, in_maps, core_ids=list(range(N_CORES)), trace=trace
    )
    _LAST_PERF["exec_time_ns"] = res.exec_time_ns
    _LAST_PERF["trace"] = (
        res.instructions_and_trace[1] if res.instructions_and_trace else None
    )
    _LAST_PERF["insts"] = (
        res.instructions_and_trace[0] if res.instructions_and_trace else None
    )
    _LAST_PERF["scopes"] = res.per_core_scope_times
    out = np.zeros((T, HIDDEN), dtype=np.float64)
    for r in res.results:
        out += r["outp"].astype(np.float64)
    return out.astype(np.float32)

